# revision 2
# baseline (speedup 1.0000x reference)
"""CBOW embedding-lookup kernel for 8 Trainium2 NeuronCores.

Math: out[b, c, :] = sum_j emb[tok[b, j, c]] with tok the +-1/+-2 shifted
tokens (OOB -> token 0).  Each output row is the full-sequence embedding sum
S[b] = sum_j emb[x[b, j]] plus tiny edge corrections:
    out[b, 0] = S - e(x[b, L-1]) + e0
    out[b, 1] = S - e(x[b, L-1]) - e(x[b, L-2]) + 2*e0
    out[b, 2] = S - e(x[b, 0])   + e0
    out[b, 3] = S - e(x[b, 0])   - e(x[b, 1])   + 2*e0
so the kernel is a pure gather+reduce: data-parallel over batch (8 rows per
core), indirect-DMA gather of 512B embedding rows from HBM (128 rows per
instruction - the HW limit for indirect DMA), PE ones-matmul accumulation
into PSUM for the per-row sums, and a small 41x32 matmul for the edge
corrections.

Layout: ids_sb[p, t] = ids_flat[128p + t], so gather chunk t (offsets
ids_sb[:, t]) puts the token from flat position 128p+t on partition p;
batch row = p // 16, handled by a block-diagonal ones lhsT in the
accumulating matmul.
"""

import os
import sys

import numpy as np

for _p in ("/opt/trn_rl_repo", "/root/.axon_site/_ro/trn_rl_repo"):
    if os.path.isdir(_p) and _p not in sys.path:
        sys.path.insert(0, _p)

B, L, V, E = 64, 2048, 100000, 128
NCORES = 8
RPC = B // NCORES        # 8 batch rows per core
TOK = RPC * L            # 16384 tokens per core
NCHUNK = TOK // 128      # 128 gather instructions per core
KU = 41                  # combine contraction: 8 S rows + 32 corr rows + e0

_CACHE = {}


def _build_mm1():
    """lhsT for the accumulating partition reduction: column m is 1 on
    partitions [16m, 16m+16) (batch row = partition // 16)."""
    mm1 = np.zeros((128, RPC), np.float32)
    for k in range(128):
        mm1[k, k // 16] = 1.0
    return mm1


def _build_w():
    """Combine matrix: out[4r+c] = sum_k W[k, 4r+c] * U[k] where
    U = [S rows 0..7 | e(x[r,0]) | e(x[r,1]) | e(x[r,L-2]) | e(x[r,L-1]) | e0]."""
    W = np.zeros((KU, 4 * RPC), np.float32)
    for r in range(RPC):
        W[r, 4 * r: 4 * r + 4] = 1.0
        W[8 + r, 4 * r + 2] = -1.0
        W[8 + r, 4 * r + 3] = -1.0
        W[16 + r, 4 * r + 3] = -1.0
        W[24 + r, 4 * r + 1] = -1.0
        W[32 + r, 4 * r + 0] = -1.0
        W[32 + r, 4 * r + 1] = -1.0
    W[40, 0::4] = 1.0
    W[40, 1::4] = 2.0
    W[40, 2::4] = 1.0
    W[40, 3::4] = 2.0
    return W


def build_nc(loop_iters=None):
    import contextlib

    import concourse.bacc as bacc
    import concourse.bass as bass
    import concourse.tile as tile
    from concourse import mybir

    f32 = mybir.dt.float32
    i32 = mybir.dt.int32

    nc = bacc.Bacc(None, target_bir_lowering=False, debug=False)
    ids = nc.dram_tensor("ids", [RPC, L], i32, kind="ExternalInput")
    emb = nc.dram_tensor("emb", [V, E], f32, kind="ExternalInput")
    mm1 = nc.dram_tensor("mm1", [128, RPC], f32, kind="ExternalInput")
    wmat = nc.dram_tensor("wmat", [KU, 4 * RPC], f32, kind="ExternalInput")
    y = nc.dram_tensor("y", [4 * RPC, E], f32, kind="ExternalOutput")

    # ids_sb[p, t] = ids_flat[128p + t]  (contiguous per-partition load)
    ids_pt = ids[:].rearrange("a b -> (a b)").rearrange("(p t) -> p t", p=128)

    with tile.TileContext(nc) as tc:
        with (
            tc.tile_pool(name="const", bufs=1) as cpool,
            tc.tile_pool(name="gat", bufs=8) as gpool,
            tc.tile_pool(name="small", bufs=1) as spool,
            tc.tile_pool(name="psum", bufs=1, space="PSUM") as ppool,
        ):
            mm1_sb = cpool.tile([128, RPC], f32)
            nc.sync.dma_start(out=mm1_sb[:], in_=mm1[:])
            w_sb = cpool.tile([KU, 4 * RPC], f32)
            nc.sync.dma_start(out=w_sb[:], in_=wmat[:])
            ids_sb = cpool.tile([128, NCHUNK], i32)
            nc.sync.dma_start(out=ids_sb[:], in_=ids_pt)

            loop_cm = (
                tc.For_i(0, loop_iters, 1)
                if loop_iters is not None
                else contextlib.nullcontext()
            )
            with loop_cm:
                psum_s = ppool.tile([RPC, E], f32)
                for t in range(NCHUNK):
                    g = gpool.tile([128, E], f32)
                    nc.gpsimd.indirect_dma_start(
                        out=g[:],
                        out_offset=None,
                        in_=emb[:],
                        in_offset=bass.IndirectOffsetOnAxis(
                            ap=ids_sb[:, t: t + 1], axis=0
                        ),
                    )
                    nc.tensor.matmul(
                        psum_s[:],
                        mm1_sb[:],
                        g[:],
                        start=(t == 0),
                        stop=(t == NCHUNK - 1),
                    )

                # corrections: gather e(x[r,0]), e(x[r,1]), e(x[r,L-2]), e(x[r,L-1]), e0
                u = spool.tile([KU, E], f32)
                ismall = spool.tile([33, 1], i32)
                for j, col in enumerate((0, 1, L - 2, L - 1)):
                    nc.sync.dma_start(
                        out=ismall[8 * j: 8 * j + 8, :], in_=ids[:, col: col + 1]
                    )
                nc.vector.memset(ismall[32:33, :], 0)
                nc.gpsimd.indirect_dma_start(
                    out=u[8:KU, :],
                    out_offset=None,
                    in_=emb[:],
                    in_offset=bass.IndirectOffsetOnAxis(ap=ismall[:, :], axis=0),
                )
                nc.vector.tensor_copy(u[0:RPC, :], psum_s[:])

                psum_o = ppool.tile([4 * RPC, E], f32)
                nc.tensor.matmul(psum_o[:], w_sb[:], u[:], start=True, stop=True)
                osb = spool.tile([4 * RPC, E], f32)
                nc.vector.tensor_copy(osb[:], psum_o[:])
                nc.sync.dma_start(out=y[:], in_=osb[:])

    nc.compile()
    return nc


def _get_nc():
    if "nc" not in _CACHE:
        _CACHE["nc"] = build_nc()
    return _CACHE["nc"]


def make_in_maps(x32, embf):
    mm1 = _build_mm1()
    wmat = _build_w()
    return [
        {
            "ids": np.ascontiguousarray(x32[c * RPC:(c + 1) * RPC]),
            "emb": embf,
            "mm1": mm1,
            "wmat": wmat,
        }
        for c in range(NCORES)
    ]


def kernel(x: np.ndarray, emb: np.ndarray) -> np.ndarray:
    from concourse.bass_utils import run_bass_kernel_spmd

    nc = _get_nc()
    x32 = np.asarray(x).astype(np.int32)
    embf = np.ascontiguousarray(np.asarray(emb, dtype=np.float32))
    res = run_bass_kernel_spmd(nc, make_in_maps(x32, embf), core_ids=list(range(NCORES)))
    out = np.concatenate(
        [np.asarray(r["y"]).reshape(RPC, 4, E) for r in res.results], axis=0
    )
    return out.astype(np.float32)


if __name__ == "__main__":
    xs = np.random.randint(0, V, size=(B, L)).astype(np.int64)
    es = np.random.randn(V, E).astype(np.float32)
    out = kernel(xs, es)
    print("out", out.shape, out.dtype)
